# revision 1
# baseline (speedup 1.0000x reference)
"""GNN message-passing (NORMADJ graph conv) on 8 Trainium2 NeuronCores.

Math (reference):
    d_e = pow(diags, e)
    gso_1[e]  = m2 * d_e2[row[e]] * d_e3[col[e]]        edge weights
    gso_2[i]  = m1*d_e1[i] + m2*d_e2[i]*d_e3[i] + m3    self-loop weights
    out[i]    = sum_{e: col[e]==i} gso_1[e] * x[row[e]] + gso_2[i] * x[i]

Key identity: d_e3[col] depends only on the destination, so
    out[i] = m2*d_e3[i] * sum_{e: col[e]==i} (d_e2[row[e]] * x[row[e]]) + gso_2[i]*x[i]
i.e. pre-scale x rows once on device (xs = d_e2 * x), scatter-add gathered xs
rows, post-scale per destination node. No per-edge weights.

Distribution: edges sharded by DESTINATION node range (8 ranges of 12544
nodes). Each core computes its own output rows exactly -> no collective.
xs is replicated per core (sources are random). Host does index packing only
(counting sort of edges by (dest tile, source bank) + padding).

Per destination tile of 128 nodes:
  - source rows gathered with gpsimd.dma_gather (f32, 256B rows). int16 index
    limit -> xs is split in 4 banks of 25088 rows; per (tile, bank) the edge
    list is padded to cap_b*128 slots (idx 0 pads, masked later). Gathers are
    grouped T_GROUP tiles per instruction to amortize SWDGE overhead.
  - ACT copy converts gathered f32 messages to bf16
  - one DVE is_equal per tile builds all one-hot selection chunks at once:
    S^T[e, i] = (col_rel[e] == i), col_rel = -1 on pad slots
  - KT accumulating matmuls: psum[128 nodes, 64] += S^T_chunk.T @ msg_chunk
  - post: out = psum * (m2*d_e3) + gso_2 * x   (per-partition scalars)
"""

import numpy as np
import ml_dtypes

P = 128
D = 64
N_CORES = 8
N_NODES = 100000
TPC = 98                      # destination tiles per core
NPC = TPC * P                 # nodes per core (12544)
NPAD = N_CORES * NPC          # padded node count (100352)
NT = NPAD // P                # total node tiles incl. padding (784)
NB = 4                        # source banks (int16 gather index limit)
BANK = NPAD // NB             # 25088 rows per bank
T_GROUP = 4                   # dest tiles per dma_gather instruction
XS_G = 16                     # node tiles per xs pre-scale group
NG = NT // XS_G               # xs groups (49)
CAPS_DEFAULT = (9, 10, 9, 9)  # chunks per (tile, bank), fitted to the data

_cache = {}


def _groups():
    gs = []
    t = 0
    while t < TPC:
        gs.append((t, min(T_GROUP, TPC - t)))
        t += min(T_GROUP, TPC - t)
    return gs


def _build_program(caps, n_cores, ablate=()):
    import concourse.bacc as bacc
    import concourse.mybir as mybir
    from concourse.tile import TileContext

    f32 = mybir.dt.float32
    bf16 = mybir.dt.bfloat16
    i16 = mybir.dt.int16
    ACT = mybir.ActivationFunctionType

    KT = sum(caps)
    groups = _groups()
    idx_cols_total = sum(tg * cb * 8 for (_, tg) in groups for cb in caps)

    nc = bacc.Bacc(
        "TRN2", target_bir_lowering=False, debug=False, num_devices=n_cores
    )

    xfull = nc.dram_tensor("xfull", [NPAD, D], f32, kind="ExternalInput")
    diagsT = nc.dram_tensor("diagsT", [P, NT], f32, kind="ExternalInput")
    ddestT = nc.dram_tensor("ddestT", [P, TPC], f32, kind="ExternalInput")
    xdest = nc.dram_tensor("xdest", [NPC, D], f32, kind="ExternalInput")
    idx16 = nc.dram_tensor("idx16", [P, idx_cols_total], i16, kind="ExternalInput")
    colrelT = nc.dram_tensor("colrelT", [P, TPC * KT], bf16, kind="ExternalInput")
    iota = nc.dram_tensor("iota", [P, KT * P], bf16, kind="ExternalInput")
    scal_in = {
        s: nc.dram_tensor(s + "c", [P, 1], f32, kind="ExternalInput")
        for s in ("m1", "m2", "m3", "e1", "e2", "e3")
    }
    out_d = nc.dram_tensor("out", [NPC, D], f32, kind="ExternalOutput")

    with TileContext(nc) as tc:
        with (
            tc.tile_pool(name="const", bufs=1) as const,
            tc.tile_pool(name="vecs", bufs=1) as vecs,
            tc.tile_pool(name="xsload", bufs=2) as xsload,
            tc.tile_pool(name="xsst", bufs=2) as xsst,
            tc.tile_pool(name="idxp", bufs=3) as idxp,
            tc.tile_pool(name="msgf", bufs=3) as msgfp,
            tc.tile_pool(name="msgb", bufs=8) as msgbp,
            tc.tile_pool(name="stp", bufs=3) as stp,
            tc.tile_pool(name="outp", bufs=4) as outp,
            tc.tile_pool(name="psum", bufs=4, space="PSUM") as psum,
            tc.tile_pool(name="dram", bufs=1, space="DRAM") as dram,
        ):
            # resident constants / index tables
            iota_sb = const.tile([P, KT * P], bf16)
            nc.sync.dma_start(out=iota_sb[:], in_=iota[:])
            colT_sb = const.tile([P, TPC * KT], bf16)
            nc.sync.dma_start(out=colT_sb[:], in_=colrelT[:])
            xdest_sb = const.tile([P, TPC * D], f32)
            nc.sync.dma_start(
                out=xdest_sb[:].rearrange("p (t d) -> p t d", d=D),
                in_=xdest[:].rearrange("(t p) d -> p t d", p=P),
            )
            scal = {}
            for s, t in scal_in.items():
                scal[s] = const.tile([P, 1], f32, name="scal_" + s)
                nc.sync.dma_start(out=scal[s][:], in_=t[:])

            # ---- per-node scalar vectors ----------------------------------
            diagsT_sb = vecs.tile([P, NT], f32)
            nc.sync.dma_start(out=diagsT_sb[:], in_=diagsT[:])
            ddestT_sb = vecs.tile([P, TPC], f32)
            nc.sync.dma_start(out=ddestT_sb[:], in_=ddestT[:])

            ln_all = vecs.tile([P, NT], f32)
            nc.scalar.activation(out=ln_all[:], in_=diagsT_sb[:], func=ACT.Ln)
            ln_dest = vecs.tile([P, TPC], f32)
            nc.scalar.activation(out=ln_dest[:], in_=ddestT_sb[:], func=ACT.Ln)

            d2_all = vecs.tile([P, NT], f32)
            nc.scalar.activation(
                out=d2_all[:], in_=ln_all[:], func=ACT.Exp, scale=scal["e2"][:, 0:1]
            )
            d1d = vecs.tile([P, TPC], f32)
            nc.scalar.activation(
                out=d1d[:], in_=ln_dest[:], func=ACT.Exp, scale=scal["e1"][:, 0:1]
            )
            d2d = vecs.tile([P, TPC], f32)
            nc.scalar.activation(
                out=d2d[:], in_=ln_dest[:], func=ACT.Exp, scale=scal["e2"][:, 0:1]
            )
            d3d = vecs.tile([P, TPC], f32)
            nc.scalar.activation(
                out=d3d[:], in_=ln_dest[:], func=ACT.Exp, scale=scal["e3"][:, 0:1]
            )
            post3 = vecs.tile([P, TPC], f32)
            nc.vector.tensor_scalar_mul(
                out=post3[:], in0=d3d[:], scalar1=scal["m2"][:, 0:1]
            )
            gso2 = vecs.tile([P, TPC], f32)
            nc.vector.tensor_mul(out=gso2[:], in0=d2d[:], in1=post3[:])
            t1 = vecs.tile([P, TPC], f32)
            nc.vector.tensor_scalar_mul(
                out=t1[:], in0=d1d[:], scalar1=scal["m1"][:, 0:1]
            )
            nc.vector.tensor_add(out=gso2[:], in0=gso2[:], in1=t1[:])
            nc.vector.tensor_scalar_add(
                out=gso2[:], in0=gso2[:], scalar1=scal["m3"][:, 0:1]
            )

            # ---- xs = d2[j] * x[j], f32 in DRAM ---------------------------
            xs_dram = dram.tile([NPAD, D], f32)
            xf_g = xfull[:].rearrange("(g k p) d -> g p k d", k=XS_G, p=P)
            xs_g = xs_dram[:].rearrange("(g k p) d -> g p k d", k=XS_G, p=P)
            for g in range(NG if "xs" not in ablate else 0):
                xg = xsload.tile([P, XS_G * D], f32)
                nc.sync.dma_start(
                    out=xg[:].rearrange("p (k d) -> p k d", d=D), in_=xf_g[g]
                )
                xsg = xsst.tile([P, XS_G * D], f32)
                nc.vector.tensor_mul(
                    out=xsg[:].rearrange("p (k d) -> p k d", d=D),
                    in0=xg[:].rearrange("p (k d) -> p k d", d=D),
                    in1=d2_all[:, g * XS_G : (g + 1) * XS_G].to_broadcast(
                        [P, XS_G, D]
                    ),
                )
                nc.sync.dma_start(
                    out=xs_g[g],
                    in_=xsg[:].rearrange("p (k d) -> p k d", d=D),
                )

            # ---- main loop: gather / select / matmul-accumulate ------------
            iota3 = iota_sb[:].rearrange("p (c q) -> p c q", q=P)
            idx_off = 0
            for (t0, tg) in groups:
                # per-group gathers (one per bank) + bf16 conversion
                gcols = sum(tg * cb * 8 for cb in caps)
                idx_sb = idxp.tile([P, gcols], i16, name="idx_sb")
                nc.sync.dma_start(
                    out=idx_sb[:], in_=idx16[:, idx_off : idx_off + gcols]
                )
                idx_off += gcols
                msgs = []
                boff = 0
                for b in range(NB):
                    cb = caps[b]
                    nidx = tg * cb * P
                    msgf = msgfp.tile([P, tg * cb * D], f32, tag="msgf")
                    nc.gpsimd.dma_gather(
                        out_ap=msgf[:].rearrange("p (k d) -> p k d", d=D),
                        in_ap=xs_dram[b * BANK : (b + 1) * BANK, :],
                        idxs_ap=idx_sb[:, boff : boff + nidx // 16],
                        num_idxs=nidx,
                        num_idxs_reg=nidx,
                        elem_size=D,
                        single_packet=False,
                    )
                    boff += nidx // 16
                    msgb = msgbp.tile([P, tg * cb * D], bf16, tag="msgb")
                    nc.scalar.copy(out=msgb[:], in_=msgf[:])
                    msgs.append(msgb)

                for q in range(tg):
                    t = t0 + q
                    sT = (
                        stp.tile([P, KT * P], bf16, name="sT")
                        if "st" not in ablate
                        else iota_sb
                    )
                    if "st" not in ablate:
                        nc.vector.tensor_tensor(
                            out=sT[:].rearrange("p (c q2) -> p c q2", q2=P),
                            in0=colT_sb[:, t * KT : (t + 1) * KT].to_broadcast(
                                [P, KT, P]
                            ),
                            in1=iota3,
                            op=mybir.AluOpType.is_equal,
                        )
                    acc = psum.tile([P, D], f32, name="acc")
                    if "mm" in ablate:
                        nc.tensor.matmul(
                            out=acc[:],
                            lhsT=sT[:, 0:P],
                            rhs=msgs[0][:, q * caps[0] * D : (q * caps[0] + 1) * D],
                            start=True,
                            stop=True,
                        )
                    else:
                        j = 0
                        for b in range(NB):
                            cb = caps[b]
                            for k in range(cb):
                                nc.tensor.matmul(
                                    out=acc[:],
                                    lhsT=sT[:, j * P : (j + 1) * P],
                                    rhs=msgs[b][:, (q * cb + k) * D : (q * cb + k + 1) * D],
                                    start=(j == 0),
                                    stop=(j == KT - 1),
                                )
                                j += 1
                    out_sb = outp.tile([P, D], f32, name="out_sb")
                    nc.vector.tensor_scalar_mul(
                        out=out_sb[:], in0=acc[:], scalar1=post3[:, t : t + 1]
                    )
                    self_sb = outp.tile([P, D], f32, name="self_sb")
                    nc.vector.tensor_scalar_mul(
                        out=self_sb[:],
                        in0=xdest_sb[:, t * D : (t + 1) * D],
                        scalar1=gso2[:, t : t + 1],
                    )
                    nc.vector.tensor_add(
                        out=out_sb[:], in0=out_sb[:], in1=self_sb[:]
                    )
                    nc.sync.dma_start(
                        out=out_d[t * P : (t + 1) * P, :], in_=out_sb[:]
                    )

    nc.compile()
    return nc


def _get_program(caps, n_cores, ablate=()):
    key = (tuple(caps), n_cores, tuple(ablate))
    if key not in _cache:
        _cache[key] = _build_program(tuple(caps), n_cores, ablate)
    return _cache[key]


def compute_caps(col, row):
    tile = col >> 7
    bank = row // BANK
    cnt = np.zeros((NT, NB), np.int64)
    np.add.at(cnt, (tile, bank), 1)
    caps = np.maximum(np.ceil(cnt.max(axis=0) / P).astype(int), 1)
    return tuple(int(max(c, d)) for c, d in zip(caps, CAPS_DEFAULT))


def pack_inputs(x, edge_index, diags, m1, m2, m3, e1, e2, e3, caps):
    """Host-side index packing. Returns list of per-core input dicts."""
    bf16 = ml_dtypes.bfloat16
    row = np.ascontiguousarray(edge_index[0]).astype(np.int64, copy=False)
    col = np.ascontiguousarray(edge_index[1]).astype(np.int64, copy=False)
    KT = sum(caps)
    bank_col_off = np.concatenate([[0], np.cumsum(caps)])  # chunk offsets

    tile = col >> 7
    bank = row // BANK
    key = tile * NB + bank
    order = np.argsort(key, kind="stable")
    row_s = row[order]
    col_s = col[order]
    key_s = key[order]

    counts = np.bincount(key_s, minlength=NT * NB)
    cnt2 = counts.reshape(NT, NB)
    assert (cnt2.max(axis=0) <= np.array(caps) * P).all(), cnt2.max(axis=0)
    starts = np.concatenate([[0], np.cumsum(counts)[:-1]])
    within = np.arange(len(row_s)) - starts[key_s]

    # slot column within the tile's KT*128 layout (bank-major chunks)
    slot = bank_col_off[key_s % NB] * P + within
    tid = key_s // NB
    idx_pad = np.zeros((NT, KT * P), np.int16)
    colrel_pad = np.full((NT, KT * P), -1.0, np.float32)
    idx_pad[tid, slot] = (row_s - (key_s % NB) * BANK).astype(np.int16)
    colrel_pad[tid, slot] = (col_s & 127).astype(np.float32)

    # colrelT: [core][p, t*KT + j]
    cr = colrel_pad.reshape(N_CORES, TPC, KT, P).transpose(0, 3, 1, 2)
    colrelT = np.ascontiguousarray(cr.reshape(N_CORES, P, TPC * KT)).astype(bf16)

    # idx16: per group g, per bank b: flat sequence i over (q, k, p);
    # value at [i % 16, base + i // 16], replicated across partition groups.
    groups = _groups()
    idx_cols_total = sum(tg * cb * 8 for (_, tg) in groups for cb in caps)
    idx16 = np.zeros((N_CORES, 16, idx_cols_total), np.int16)
    idx_pad_c = idx_pad.reshape(N_CORES, TPC, KT * P)
    for c in range(N_CORES):
        base = 0
        for (t0, tg) in groups:
            for b in range(NB):
                cb = caps[b]
                nidx = tg * cb * P
                seq = idx_pad_c[
                    c, t0 : t0 + tg, bank_col_off[b] * P : bank_col_off[b + 1] * P
                ].reshape(nidx)
                idx16[c, :, base : base + nidx // 16] = seq.reshape(
                    nidx // 16, 16
                ).T
                base += nidx // 16
        assert base == idx_cols_total
    idx16 = np.ascontiguousarray(
        np.broadcast_to(
            idx16[:, None, :, :], (N_CORES, 8, 16, idx_cols_total)
        ).reshape(N_CORES, P, idx_cols_total)
    )

    n = x.shape[0]
    xbig = np.zeros((NPAD, D), np.float32)
    xbig[:n] = x
    dbig = np.ones(NPAD, np.float32)
    dbig[:n] = diags
    diagsT_h = np.ascontiguousarray(dbig.reshape(NT, P).T)
    iota_h = np.ascontiguousarray(
        np.broadcast_to(
            np.tile(np.arange(P, dtype=np.float32), KT)[None, :], (P, KT * P)
        )
    ).astype(bf16)

    scal_h = {
        "m1c": m1, "m2c": m2, "m3c": m3, "e1c": e1, "e2c": e2, "e3c": e3,
    }
    scal_h = {
        k: np.full((P, 1), np.float32(np.asarray(v).reshape(-1)[0]))
        for k, v in scal_h.items()
    }

    in_maps = []
    for k in range(N_CORES):
        lo, hi = k * NPC, (k + 1) * NPC
        in_maps.append(
            {
                "xfull": xbig,
                "diagsT": diagsT_h,
                "ddestT": np.ascontiguousarray(dbig[lo:hi].reshape(TPC, P).T),
                "xdest": xbig[lo:hi],
                "idx16": idx16[k],
                "colrelT": colrelT[k],
                "iota": iota_h,
                **scal_h,
            }
        )
    return in_maps


def kernel(x, edge_index, edge_index_id=None, diags=None, m1=None, m2=None,
           m3=None, e1=None, e2=None, e3=None, a=None, **_):
    from concourse.bass_utils import run_bass_kernel_spmd

    x = np.ascontiguousarray(np.asarray(x, dtype=np.float32))
    edge_index = np.asarray(edge_index)
    caps = compute_caps(
        edge_index[1].astype(np.int64), edge_index[0].astype(np.int64)
    )
    in_maps = pack_inputs(
        x, edge_index, np.asarray(diags, dtype=np.float32),
        m1, m2, m3, e1, e2, e3, caps,
    )
    nc = _get_program(caps, N_CORES)
    res = run_bass_kernel_spmd(nc, in_maps, list(range(N_CORES)))
    out = np.concatenate([res.results[k]["out"] for k in range(N_CORES)], axis=0)
    return np.ascontiguousarray(out[:N_NODES])

